# revision 3
# baseline (speedup 1.0000x reference)
"""GraphSAGE 3-layer kernel for 8 trn2 NeuronCores, v2.

Sharding: dst-node parallel (6250 dst nodes per core, padded to 6272).
Per layer: bulk dma_gather of x[src] rows (bf16, chunked, split into two
src-range streams so indices fit int16), segment-sum via one-hot matmul
into PSUM (feature-major, bf16 operands), two GEMMs (Wl@mean + Wr@x),
BN stats via AllReduce, BN+ReLU on Act engine, PE transpose to
node-major, AllGather (bf16) into the next layer's gather table.
"""
import numpy as np

N = 50000
E = 800000
IN_DIM = 128
HID = 128
OUT_DIM = 64
BN_EPS = 1e-5
NC_ = 8
NPC = 6250            # real nodes per core
PADC = 6272           # padded nodes per core (49*128)
R = NC_ * PADC        # gather table rows (50176)
HALF = R // 2         # 25088: src-range split so idx fits int16
NB = PADC // 128      # dst blocks per core (49)
P = 128
K = 8                 # one-hot tiles built per DVE instruction
CHB = 7               # dst blocks per gather chunk

ZERO_A = NPC                      # a known zero row in [0, HALF)
ZERO_B = 5 * PADC + NPC - HALF    # a known zero row in [HALF, R), rel offset


def _remap(n):
    return (n // NPC) * PADC + (n % NPC)


def _preprocess(x, edge_index):
    import ml_dtypes

    src = edge_index[0].astype(np.int64)
    dst = edge_index[1].astype(np.int64)
    deg = np.bincount(dst, minlength=N).astype(np.float32)
    recip = (1.0 / np.maximum(deg, 1.0)).astype(np.float32)
    src_r = _remap(src)

    # per core: sort edges by (block, src-range) key; slice per group
    kA = np.zeros((NC_, NB), np.int64)
    kB = np.zeros((NC_, NB), np.int64)
    groups = []  # [c] -> (s_sorted, db_sorted, offsets[2*NB+1])
    for c in range(NC_):
        m = (dst // NPC) == c
        s = src_r[m]
        dl = dst[m] - c * NPC
        key = (dl // P) * 2 + (s >= HALF)
        order = np.argsort(key, kind="stable")
        s_s, dl_s, key_s = s[order], dl[order], key[order]
        cnt = np.bincount(key_s, minlength=2 * NB)
        offs = np.concatenate([[0], np.cumsum(cnt)])
        kA[c] = cnt[0::2]
        kB[c] = cnt[1::2]
        groups.append((s_s, (dl_s % P).astype(np.float32), offs))

    nTA = np.ceil(kA / P).astype(np.int64).max(axis=0)
    nTB = np.ceil(kB / P).astype(np.int64).max(axis=0)
    tAOff = np.concatenate([[0], np.cumsum(nTA)]).astype(np.int64)
    tBOff = np.concatenate([[0], np.cumsum(nTB)]).astype(np.int64)
    TA, TB = int(tAOff[-1]), int(tBOff[-1])
    totT = TA + TB
    assert np.all(nTA + nTB >= 1)

    idxA = np.full((NC_, TA * P), ZERO_A, np.int16)
    dlA = np.zeros((NC_, TA * P), np.float32)
    idxB = np.full((NC_, TB * P), ZERO_B, np.int16)
    dlB = np.zeros((NC_, TB * P), np.float32)
    for c in range(NC_):
        s_s, db_s, offs = groups[c]
        for b in range(NB):
            a0, a1 = offs[2 * b], offs[2 * b + 1]
            o = tAOff[b] * P
            idxA[c, o:o + (a1 - a0)] = s_s[a0:a1]
            dlA[c, o:o + (a1 - a0)] = db_s[a0:a1]
            b0_, b1_ = offs[2 * b + 1], offs[2 * b + 2]
            o = tBOff[b] * P
            idxB[c, o:o + (b1_ - b0_)] = s_s[b0_:b1_] - HALF
            dlB[c, o:o + (b1_ - b0_)] = db_s[b0_:b1_]

    def wrap(flat):
        # [NC, L] -> [NC, 128, L//16]; idx i at [i%16 + 16k, i//16]
        t = flat.reshape(NC_, -1, 16).transpose(0, 2, 1)
        return np.ascontiguousarray(np.tile(t, (1, 8, 1)))

    idxA_w = wrap(idxA)
    idxB_w = wrap(idxB)
    dl_all = np.concatenate(
        [dlA.reshape(NC_, TA, P).transpose(0, 2, 1),
         dlB.reshape(NC_, TB, P).transpose(0, 2, 1)], axis=2)
    dl_all = np.ascontiguousarray(dl_all).astype(ml_dtypes.bfloat16)

    recip_b = np.zeros((NC_, P, PADC), np.float32)
    for c in range(NC_):
        recip_b[c, :, :NPC] = recip[c * NPC:(c + 1) * NPC][None, :]
    recip_b = recip_b.astype(ml_dtypes.bfloat16)

    x_table = np.zeros((R, IN_DIM), np.float32)
    for c in range(NC_):
        x_table[c * PADC:c * PADC + NPC] = x[c * NPC:(c + 1) * NPC]
    x_table = x_table.astype(ml_dtypes.bfloat16)
    xT_own = np.zeros((NC_, P, PADC), np.float32)
    for c in range(NC_):
        xT_own[c, :, :NPC] = x[c * NPC:(c + 1) * NPC].T
    xT_own = xT_own.astype(ml_dtypes.bfloat16)

    plan = (nTA, nTB, tAOff, tBOff, TA, TB, totT)
    return idxA_w, idxB_w, dl_all, recip_b, x_table, xT_own, plan


def _build(plan, n_layers=3, dbg_hpre=False):
    import contextlib
    import concourse.bass as bass  # noqa: F401
    import concourse.bacc as bacc
    import concourse.tile as tile
    from concourse import mybir
    from concourse.masks import make_identity

    nTA, nTB, tAOff, tBOff, TA, TB, totT = plan
    f32 = mybir.dt.float32
    bf16 = mybir.dt.bfloat16
    i16 = mybir.dt.int16
    AF = mybir.ActivationFunctionType

    nc = bacc.Bacc("TRN2")
    t_tab = nc.dram_tensor("x_table", [R, IN_DIM], bf16, kind="ExternalInput")
    t_idxA = nc.dram_tensor("idxA", [P, TA * 8], i16, kind="ExternalInput")
    t_idxB = nc.dram_tensor("idxB", [P, TB * 8], i16, kind="ExternalInput")
    t_dl = nc.dram_tensor("dl", [P, totT], bf16, kind="ExternalInput")
    t_rc = nc.dram_tensor("recip", [P, PADC], bf16, kind="ExternalInput")
    t_xT = nc.dram_tensor("xT_own", [P, PADC], bf16, kind="ExternalInput")
    t_w = {}
    for l, fo in ((1, HID), (2, HID), (3, OUT_DIM)):
        t_w[f"wl{l}"] = nc.dram_tensor(f"wl{l}", [P, fo], bf16, kind="ExternalInput")
        t_w[f"wr{l}"] = nc.dram_tensor(f"wr{l}", [P, fo], bf16, kind="ExternalInput")
    t_bn = nc.dram_tensor("bn", [P, 4], f32, kind="ExternalInput")  # g1 be1 g2 be2
    t_b3 = nc.dram_tensor("b3", [P, 1], f32, kind="ExternalInput")
    t_out = nc.dram_tensor("out", [PADC, OUT_DIM], f32, kind="ExternalOutput")
    t_dbg = nc.dram_tensor("dbg", [P, PADC], f32, kind="ExternalOutput") if dbg_hpre else None

    h_own = [nc.dram_tensor(f"h_own{l}", [PADC, HID], bf16, kind="Internal") for l in (1, 2)]
    h_tab = [nc.dram_tensor(f"h_tab{l}", [R, HID], bf16, kind="Internal", addr_space="Shared") for l in (1, 2)]
    st_in = [nc.dram_tensor(f"st_in{l}", [P, 2], f32, kind="Internal") for l in (1, 2)]
    st_out = [nc.dram_tensor(f"st_out{l}", [P, 2], f32, kind="Internal", addr_space="Shared") for l in (1, 2)]
    RG = [list(range(NC_))]

    chunks = [(b0, min(b0 + CHB, NB)) for b0 in range(0, NB, CHB)]

    with tile.TileContext(nc) as tc:
        with contextlib.ExitStack() as ctx:
            singles = ctx.enter_context(tc.tile_pool(name="singles", bufs=1))
            gp = ctx.enter_context(tc.tile_pool(name="g", bufs=2))
            ohp = ctx.enter_context(tc.tile_pool(name="oh", bufs=6))
            mp = ctx.enter_context(tc.tile_pool(name="mp", bufs=8))
            tsp = ctx.enter_context(tc.tile_pool(name="ts", bufs=4))
            pseg = ctx.enter_context(tc.tile_pool(name="pseg", bufs=4, space="PSUM"))
            pgem = ctx.enter_context(tc.tile_pool(name="pgem", bufs=2, space="PSUM"))
            ptr = ctx.enter_context(tc.tile_pool(name="ptr", bufs=2, space="PSUM"))

            idxA_sb = singles.tile([P, TA * 8], i16)
            idxB_sb = singles.tile([P, TB * 8], i16)
            dl_sb = singles.tile([P, totT], bf16)
            rc_sb = singles.tile([P, PADC], bf16)
            xT_sb = singles.tile([P, PADC], bf16)
            nc.sync.dma_start(out=idxA_sb[:], in_=t_idxA[:])
            nc.sync.dma_start(out=idxB_sb[:], in_=t_idxB[:])
            nc.sync.dma_start(out=dl_sb[:], in_=t_dl[:])
            nc.sync.dma_start(out=rc_sb[:], in_=t_rc[:])
            nc.sync.dma_start(out=xT_sb[:], in_=t_xT[:])
            w_sb = {}
            for k, t in t_w.items():
                w_sb[k] = singles.tile([P, t.shape[1]], bf16, name=f"w_{k}", tag=f"w_{k}")
                nc.sync.dma_start(out=w_sb[k][:], in_=t[:])
            bn_sb = singles.tile([P, 4], f32)
            nc.sync.dma_start(out=bn_sb[:], in_=t_bn[:])
            b3_sb = singles.tile([P, 1], f32)
            nc.sync.dma_start(out=b3_sb[:], in_=t_b3[:])

            ident_bf = singles.tile([P, P], bf16)
            make_identity(nc, ident_bf[:])
            ident_f = singles.tile([P, P], f32)
            make_identity(nc, ident_f[:])
            iota_i = singles.tile([P, P], mybir.dt.int32)
            nc.gpsimd.iota(iota_i[:], pattern=[[1, P]], base=0, channel_multiplier=0)
            iota_bf = singles.tile([P, P], bf16)
            nc.vector.tensor_copy(out=iota_bf[:], in_=iota_i[:])
            iota_rep = singles.tile([P, K, P], bf16)
            nc.vector.tensor_copy(out=iota_rep[:], in_=iota_bf[:].unsqueeze(1).to_broadcast([P, K, P]))
            eps_sb = singles.tile([P, 1], f32)
            nc.vector.memset(eps_sb[:], BN_EPS)

            agg = singles.tile([P, PADC], bf16)
            hpre = singles.tile([P, PADC], f32)
            hbn = singles.tile([P, PADC], bf16)
            nc.vector.memset(hpre[:], 0.0)
            nc.vector.memset(hbn[:], 0.0)

            for layer in (1, 2, 3)[:n_layers]:
                fo = OUT_DIM if layer == 3 else HID
                tab = t_tab if layer == 1 else h_tab[layer - 2]
                xTc = xT_sb if layer == 1 else hbn
                for (cb0, cb1) in chunks:
                    a0, a1 = int(tAOff[cb0]), int(tAOff[cb1])
                    bb0, bb1 = int(tBOff[cb0]), int(tBOff[cb1])
                    nA, nB_ = a1 - a0, bb1 - bb0
                    g = gp.tile([P, (nA + nB_) * P], bf16, tag="g")
                    GT = 8  # max tiles (1024 idxs) per dma_gather instruction
                    for k in range(0, nA, GT):
                        w = min(GT, nA - k)
                        nc.gpsimd.dma_gather(
                            g[:, (k) * P:(k + w) * P].rearrange("p (t f) -> p t f", f=P),
                            tab[0:HALF, :], idxA_sb[:, (a0 + k) * 8:(a0 + k + w) * 8],
                            w * P, w * P, P)
                    for k in range(0, nB_, GT):
                        w = min(GT, nB_ - k)
                        nc.gpsimd.dma_gather(
                            g[:, (nA + k) * P:(nA + k + w) * P].rearrange("p (t f) -> p t f", f=P),
                            tab[HALF:R, :], idxB_sb[:, (bb0 + k) * 8:(bb0 + k + w) * 8],
                            w * P, w * P, P)

                    # one-hot group cache for this chunk
                    ohcache = {}

                    def get_oh(dc, rs, re):
                        base = rs + ((dc - rs) // K) * K
                        w = min(K, re - base)
                        if base not in ohcache:
                            t = ohp.tile([P, w * P], bf16, tag="oh")
                            nc.vector.tensor_tensor(
                                out=t[:].rearrange("p (a b) -> p a b", b=P),
                                in0=dl_sb[:, base:base + w].unsqueeze(2).to_broadcast([P, w, P]),
                                in1=iota_rep[:, :w, :],
                                op=mybir.AluOpType.is_equal)
                            ohcache[base] = t
                        t = ohcache[base]
                        sl = dc - base
                        return t[:, sl * P:(sl + 1) * P]

                    for b in range(cb0, cb1):
                        tiles = []
                        for ti in range(int(nTA[b])):
                            gcol = int(tAOff[b]) + ti - a0
                            dcol = int(tAOff[b]) + ti
                            tiles.append((gcol, dcol, a0, a1))
                        for ti in range(int(nTB[b])):
                            gcol = nA + int(tBOff[b]) + ti - bb0
                            dcol = TA + int(tBOff[b]) + ti
                            tiles.append((gcol, dcol, TA + bb0, TA + bb1))
                        ps = pseg.tile([P, P], f32, tag="ps")
                        nt = len(tiles)
                        for j, (gcol, dcol, rs, re) in enumerate(tiles):
                            oh = get_oh(dcol, rs, re)
                            nc.tensor.matmul(
                                out=ps[:], lhsT=g[:, gcol * P:(gcol + 1) * P],
                                rhs=oh, start=(j == 0), stop=(j == nt - 1))
                        nc.vector.tensor_tensor(
                            out=agg[:, b * P:(b + 1) * P], in0=ps[:],
                            in1=rc_sb[:, b * P:(b + 1) * P], op=mybir.AluOpType.mult)

                    # GEMMs for this chunk's columns
                    c_off = cb0 * P
                    c_w = (cb1 - cb0) * P
                    for off, w in ((c_off, c_w // 2), (c_off + c_w // 2, c_w - c_w // 2)):
                        pg = pgem.tile([P, 448], f32, tag="pg")
                        nc.tensor.matmul(out=pg[:fo, :w], lhsT=w_sb[f"wl{layer}"][:],
                                         rhs=agg[:, off:off + w], start=True, stop=False)
                        nc.tensor.matmul(out=pg[:fo, :w], lhsT=w_sb[f"wr{layer}"][:],
                                         rhs=xTc[:, off:off + w], start=False, stop=True)
                        if layer == 3:
                            nc.scalar.activation(out=hpre[:fo, off:off + w], in_=pg[:fo, :w],
                                                 func=AF.Identity, bias=b3_sb[:fo], scale=1.0)
                        else:
                            nc.scalar.activation(out=hpre[:, off:off + w], in_=pg[:, :w],
                                                 func=AF.Copy, bias=0.0, scale=1.0)

                if dbg_hpre and layer == n_layers:
                    nc.sync.dma_start(out=t_dbg[:], in_=hpre[:])
                    break
                if layer < 3:
                    li = layer - 1
                    s1 = mp.tile([P, 1], f32, tag="st")
                    nc.vector.tensor_reduce(out=s1[:], in_=hpre[:], axis=mybir.AxisListType.X,
                                            op=mybir.AluOpType.add)
                    s2 = mp.tile([P, 1], f32, tag="st")
                    nc.scalar.activation(out=agg[:], in_=hpre[:], func=AF.Square,
                                         bias=0.0, scale=1.0, accum_out=s2[:])
                    stt = mp.tile([P, 2], f32, tag="st2")
                    nc.vector.tensor_copy(out=stt[:, 0:1], in_=s1[:])
                    nc.vector.tensor_copy(out=stt[:, 1:2], in_=s2[:])
                    nc.sync.dma_start(out=st_in[li][:], in_=stt[:])
                    nc.gpsimd.collective_compute(
                        "AllReduce", mybir.AluOpType.add, replica_groups=RG,
                        ins=[st_in[li][:]], outs=[st_out[li][:]])
                    str_ = mp.tile([P, 2], f32, tag="st2")
                    nc.sync.dma_start(out=str_[:], in_=st_out[li][:])
                    mu = mp.tile([P, 1], f32, tag="st")
                    nc.scalar.mul(out=mu[:], in_=str_[:, 0:1], mul=1.0 / N)
                    ex2 = mp.tile([P, 1], f32, tag="st")
                    nc.scalar.mul(out=ex2[:], in_=str_[:, 1:2], mul=1.0 / N)
                    var = mp.tile([P, 1], f32, tag="st")
                    nc.vector.tensor_tensor(out=var[:], in0=mu[:], in1=mu[:], op=mybir.AluOpType.mult)
                    nc.vector.tensor_tensor(out=var[:], in0=ex2[:], in1=var[:], op=mybir.AluOpType.subtract)
                    rs_ = mp.tile([P, 1], f32, tag="st")
                    nc.scalar.activation(out=rs_[:], in_=var[:], func=AF.Sqrt,
                                         bias=eps_sb[:], scale=1.0, alpha=0.0)
                    nc.vector.reciprocal(out=rs_[:], in_=rs_[:])
                    a_t = mp.tile([P, 1], f32, tag="st")
                    nc.vector.tensor_tensor(out=a_t[:], in0=rs_[:], in1=bn_sb[:, 2 * li:2 * li + 1],
                                            op=mybir.AluOpType.mult)
                    bi = mp.tile([P, 1], f32, tag="st")
                    nc.vector.tensor_tensor(out=bi[:], in0=mu[:], in1=a_t[:], op=mybir.AluOpType.mult)
                    nc.vector.tensor_tensor(out=bi[:], in0=bn_sb[:, 2 * li + 1:2 * li + 2], in1=bi[:],
                                            op=mybir.AluOpType.subtract)
                    nc.scalar.activation(out=hbn[:], in_=hpre[:], func=AF.Relu,
                                         bias=bi[:], scale=a_t[:])
                    nc.vector.memset(hbn[:, NPC:PADC], 0.0)
                    for k in range(NB):
                        pt = ptr.tile([P, P], bf16, tag="pt")
                        nc.tensor.transpose(out=pt[:], in_=hbn[:, k * P:(k + 1) * P], identity=ident_bf[:])
                        ts_ = tsp.tile([P, P], bf16, tag="ts")
                        nc.scalar.activation(out=ts_[:], in_=pt[:], func=AF.Copy, bias=0.0, scale=1.0)
                        nc.sync.dma_start(out=h_own[li][k * P:(k + 1) * P, :], in_=ts_[:])
                    nc.gpsimd.collective_compute(
                        "AllGather", mybir.AluOpType.bypass, replica_groups=RG,
                        ins=[h_own[li][:]], outs=[h_tab[li][:]])
                else:
                    for k in range(NB):
                        pt = ptr.tile([P, P], f32, tag="pt")
                        nc.tensor.transpose(out=pt[:], in_=hpre[:, k * P:(k + 1) * P], identity=ident_f[:])
                        ts_ = tsp.tile([P, P], f32, tag="ts")
                        nc.scalar.activation(out=ts_[:], in_=pt[:], func=AF.Copy, bias=0.0, scale=1.0)
                        nc.sync.dma_start(out=t_out[k * P:(k + 1) * P, :], in_=ts_[:, :OUT_DIM])
    nc.compile()
    return nc


def kernel(**inputs):
    import os
    os.environ.setdefault("BASS_NEVER_TRACE", "1")
    import ml_dtypes
    from concourse.bass_utils import run_bass_kernel_spmd

    x = np.asarray(inputs["x"], dtype=np.float32)
    ei = np.asarray(inputs["edge_index"])
    idxA_w, idxB_w, dl_all, recip_b, x_table, xT_own, plan = _preprocess(x, ei)
    nc = _build(plan)

    bn = np.stack([np.asarray(inputs["g1"]), np.asarray(inputs["be1"]),
                   np.asarray(inputs["g2"]), np.asarray(inputs["be2"])], axis=1).astype(np.float32)
    b3 = np.zeros((P, 1), np.float32)
    b3[:OUT_DIM, 0] = np.asarray(inputs["b3"], dtype=np.float32)
    wm = {}
    for l, (wl, wr) in {1: ("Wl1", "Wr1"), 2: ("Wl2", "Wr2"), 3: ("Wl3", "Wr3")}.items():
        wm[f"wl{l}"] = np.ascontiguousarray(
            np.asarray(inputs[wl], dtype=np.float32).T).astype(ml_dtypes.bfloat16)
        wm[f"wr{l}"] = np.ascontiguousarray(
            np.asarray(inputs[wr], dtype=np.float32).T).astype(ml_dtypes.bfloat16)

    in_maps = []
    for c in range(NC_):
        m = {"x_table": x_table, "idxA": idxA_w[c], "idxB": idxB_w[c],
             "dl": dl_all[c], "recip": recip_b[c], "xT_own": xT_own[c],
             "bn": bn, "b3": b3}
        m.update(wm)
        in_maps.append(m)
    res = run_bass_kernel_spmd(nc, in_maps, core_ids=list(range(NC_)))
    global _last_res
    _last_res = res
    out = np.concatenate([res.results[c]["out"][:NPC] for c in range(NC_)], axis=0)
    return out.astype(np.float32)


_last_res = None


# revision 5
# speedup vs baseline: 1.0118x; 1.0118x over previous
"""GraphSAGE 3-layer kernel for 8 trn2 NeuronCores, v2.

Sharding: dst-node parallel (6250 dst nodes per core, padded to 6272).
Per layer: bulk dma_gather of x[src] rows (bf16, chunked, split into two
src-range streams so indices fit int16), segment-sum via one-hot matmul
into PSUM (feature-major, bf16 operands), two GEMMs (Wl@mean + Wr@x),
BN stats via AllReduce, BN+ReLU on Act engine, PE transpose to
node-major, AllGather (bf16) into the next layer's gather table.
"""
import numpy as np

N = 50000
E = 800000
IN_DIM = 128
HID = 128
OUT_DIM = 64
BN_EPS = 1e-5
NC_ = 8
NPC = 6250            # real nodes per core
PADC = 6272           # padded nodes per core (49*128)
R = NC_ * PADC        # gather table rows (50176)
HALF = R // 2         # 25088: src-range split so idx fits int16
NB = PADC // 128      # dst blocks per core (49)
P = 128
K = 8                 # one-hot tiles built per DVE instruction
CHB = 5               # dst blocks per gather chunk

ZERO_A = NPC                      # a known zero row in [0, HALF)
ZERO_B = 5 * PADC + NPC - HALF    # a known zero row in [HALF, R), rel offset


def _remap(n):
    return (n // NPC) * PADC + (n % NPC)


def _preprocess(x, edge_index):
    import ml_dtypes

    src = edge_index[0].astype(np.int64)
    dst = edge_index[1].astype(np.int64)
    deg = np.bincount(dst, minlength=N).astype(np.float32)
    recip = (1.0 / np.maximum(deg, 1.0)).astype(np.float32)
    src_r = _remap(src)

    # per core: sort edges by (block, src-range) key; slice per group
    kA = np.zeros((NC_, NB), np.int64)
    kB = np.zeros((NC_, NB), np.int64)
    groups = []  # [c] -> (s_sorted, db_sorted, offsets[2*NB+1])
    for c in range(NC_):
        m = (dst // NPC) == c
        s = src_r[m]
        dl = dst[m] - c * NPC
        key = (dl // P) * 2 + (s >= HALF)
        order = np.argsort(key, kind="stable")
        s_s, dl_s, key_s = s[order], dl[order], key[order]
        cnt = np.bincount(key_s, minlength=2 * NB)
        offs = np.concatenate([[0], np.cumsum(cnt)])
        kA[c] = cnt[0::2]
        kB[c] = cnt[1::2]
        groups.append((s_s, (dl_s % P).astype(np.float32), offs))

    nTA = np.ceil(kA / P).astype(np.int64).max(axis=0)
    nTB = np.ceil(kB / P).astype(np.int64).max(axis=0)
    tAOff = np.concatenate([[0], np.cumsum(nTA)]).astype(np.int64)
    tBOff = np.concatenate([[0], np.cumsum(nTB)]).astype(np.int64)
    TA, TB = int(tAOff[-1]), int(tBOff[-1])
    totT = TA + TB
    assert np.all(nTA + nTB >= 1)

    idxA = np.full((NC_, TA * P), ZERO_A, np.int16)
    dlA = np.zeros((NC_, TA * P), np.float32)
    idxB = np.full((NC_, TB * P), ZERO_B, np.int16)
    dlB = np.zeros((NC_, TB * P), np.float32)
    for c in range(NC_):
        s_s, db_s, offs = groups[c]
        for b in range(NB):
            a0, a1 = offs[2 * b], offs[2 * b + 1]
            o = tAOff[b] * P
            idxA[c, o:o + (a1 - a0)] = s_s[a0:a1]
            dlA[c, o:o + (a1 - a0)] = db_s[a0:a1]
            b0_, b1_ = offs[2 * b + 1], offs[2 * b + 2]
            o = tBOff[b] * P
            idxB[c, o:o + (b1_ - b0_)] = s_s[b0_:b1_] - HALF
            dlB[c, o:o + (b1_ - b0_)] = db_s[b0_:b1_]

    def wrap(flat):
        # [NC, L] -> [NC, 128, L//16]; idx i at [i%16 + 16k, i//16]
        t = flat.reshape(NC_, -1, 16).transpose(0, 2, 1)
        return np.ascontiguousarray(np.tile(t, (1, 8, 1)))

    idxA_w = wrap(idxA)
    idxB_w = wrap(idxB)
    dl_all = np.concatenate(
        [dlA.reshape(NC_, TA, P).transpose(0, 2, 1),
         dlB.reshape(NC_, TB, P).transpose(0, 2, 1)], axis=2)
    dl_all = np.ascontiguousarray(dl_all).astype(ml_dtypes.bfloat16)

    recip_b = np.zeros((NC_, P, PADC), np.float32)
    for c in range(NC_):
        recip_b[c, :, :NPC] = recip[c * NPC:(c + 1) * NPC][None, :]
    recip_b = recip_b.astype(ml_dtypes.bfloat16)

    x_table = np.zeros((R, IN_DIM), np.float32)
    for c in range(NC_):
        x_table[c * PADC:c * PADC + NPC] = x[c * NPC:(c + 1) * NPC]
    x_table = x_table.astype(ml_dtypes.bfloat16)
    xT_own = np.zeros((NC_, P, PADC), np.float32)
    for c in range(NC_):
        xT_own[c, :, :NPC] = x[c * NPC:(c + 1) * NPC].T
    xT_own = xT_own.astype(ml_dtypes.bfloat16)

    plan = (nTA, nTB, tAOff, tBOff, TA, TB, totT)
    return idxA_w, idxB_w, dl_all, recip_b, x_table, xT_own, plan


def _build(plan, n_layers=3, dbg_hpre=False):
    import contextlib
    import concourse.bass as bass  # noqa: F401
    import concourse.bacc as bacc
    import concourse.tile as tile
    from concourse import mybir
    from concourse.masks import make_identity

    nTA, nTB, tAOff, tBOff, TA, TB, totT = plan
    f32 = mybir.dt.float32
    bf16 = mybir.dt.bfloat16
    i16 = mybir.dt.int16
    AF = mybir.ActivationFunctionType

    nc = bacc.Bacc("TRN2")
    t_tab = nc.dram_tensor("x_table", [R, IN_DIM], bf16, kind="ExternalInput")
    t_idxA = nc.dram_tensor("idxA", [P, TA * 8], i16, kind="ExternalInput")
    t_idxB = nc.dram_tensor("idxB", [P, TB * 8], i16, kind="ExternalInput")
    t_dl = nc.dram_tensor("dl", [P, totT], bf16, kind="ExternalInput")
    t_rc = nc.dram_tensor("recip", [P, PADC], bf16, kind="ExternalInput")
    t_xT = nc.dram_tensor("xT_own", [P, PADC], bf16, kind="ExternalInput")
    t_w = {}
    for l, fo in ((1, HID), (2, HID), (3, OUT_DIM)):
        t_w[f"wl{l}"] = nc.dram_tensor(f"wl{l}", [P, fo], bf16, kind="ExternalInput")
        t_w[f"wr{l}"] = nc.dram_tensor(f"wr{l}", [P, fo], bf16, kind="ExternalInput")
    t_bn = nc.dram_tensor("bn", [P, 4], f32, kind="ExternalInput")  # g1 be1 g2 be2
    t_b3 = nc.dram_tensor("b3", [P, 1], f32, kind="ExternalInput")
    t_out = nc.dram_tensor("out", [PADC, OUT_DIM], f32, kind="ExternalOutput")
    t_dbg = nc.dram_tensor("dbg", [P, PADC], f32, kind="ExternalOutput") if dbg_hpre else None

    h_own = [nc.dram_tensor(f"h_own{l}", [PADC, HID], bf16, kind="Internal") for l in (1, 2)]
    h_tab = [nc.dram_tensor(f"h_tab{l}", [R, HID], bf16, kind="Internal", addr_space="Shared") for l in (1, 2)]
    st_in = [nc.dram_tensor(f"st_in{l}", [P, 2], f32, kind="Internal") for l in (1, 2)]
    st_out = [nc.dram_tensor(f"st_out{l}", [P, 2], f32, kind="Internal", addr_space="Shared") for l in (1, 2)]
    RG = [list(range(NC_))]

    chunks = [(b0, min(b0 + CHB, NB)) for b0 in range(0, NB, CHB)]

    with tile.TileContext(nc) as tc:
        with contextlib.ExitStack() as ctx:
            singles = ctx.enter_context(tc.tile_pool(name="singles", bufs=1))
            gp = ctx.enter_context(tc.tile_pool(name="g", bufs=3))
            ohp = ctx.enter_context(tc.tile_pool(name="oh", bufs=6))
            mp = ctx.enter_context(tc.tile_pool(name="mp", bufs=8))
            tsp = ctx.enter_context(tc.tile_pool(name="ts", bufs=4))
            pseg = ctx.enter_context(tc.tile_pool(name="pseg", bufs=4, space="PSUM"))
            pgem = ctx.enter_context(tc.tile_pool(name="pgem", bufs=2, space="PSUM"))
            ptr = ctx.enter_context(tc.tile_pool(name="ptr", bufs=2, space="PSUM"))

            idxA_sb = singles.tile([P, TA * 8], i16)
            idxB_sb = singles.tile([P, TB * 8], i16)
            dl_sb = singles.tile([P, totT], bf16)
            rc_sb = singles.tile([P, PADC], bf16)
            xT_sb = singles.tile([P, PADC], bf16)
            nc.sync.dma_start(out=idxA_sb[:], in_=t_idxA[:])
            nc.sync.dma_start(out=idxB_sb[:], in_=t_idxB[:])
            nc.sync.dma_start(out=dl_sb[:], in_=t_dl[:])
            nc.sync.dma_start(out=rc_sb[:], in_=t_rc[:])
            nc.sync.dma_start(out=xT_sb[:], in_=t_xT[:])
            w_sb = {}
            for k, t in t_w.items():
                w_sb[k] = singles.tile([P, t.shape[1]], bf16, name=f"w_{k}", tag=f"w_{k}")
                nc.sync.dma_start(out=w_sb[k][:], in_=t[:])
            bn_sb = singles.tile([P, 4], f32)
            nc.sync.dma_start(out=bn_sb[:], in_=t_bn[:])
            b3_sb = singles.tile([P, 1], f32)
            nc.sync.dma_start(out=b3_sb[:], in_=t_b3[:])

            ident_bf = singles.tile([P, P], bf16)
            make_identity(nc, ident_bf[:])
            ident_f = singles.tile([P, P], f32)
            make_identity(nc, ident_f[:])
            iota_i = singles.tile([P, P], mybir.dt.int32)
            nc.gpsimd.iota(iota_i[:], pattern=[[1, P]], base=0, channel_multiplier=0)
            iota_bf = singles.tile([P, P], bf16)
            nc.vector.tensor_copy(out=iota_bf[:], in_=iota_i[:])
            iota_rep = singles.tile([P, K, P], bf16)
            nc.vector.tensor_copy(out=iota_rep[:], in_=iota_bf[:].unsqueeze(1).to_broadcast([P, K, P]))
            eps_sb = singles.tile([P, 1], f32)
            nc.vector.memset(eps_sb[:], BN_EPS)

            agg = singles.tile([P, PADC], bf16)
            hpre = singles.tile([P, PADC], f32)
            hbn = singles.tile([P, PADC], bf16)
            nc.vector.memset(hpre[:], 0.0)
            nc.vector.memset(hbn[:], 0.0)

            for layer in (1, 2, 3)[:n_layers]:
                fo = OUT_DIM if layer == 3 else HID
                tab = t_tab if layer == 1 else h_tab[layer - 2]
                xTc = xT_sb if layer == 1 else hbn
                for (cb0, cb1) in chunks:
                    a0, a1 = int(tAOff[cb0]), int(tAOff[cb1])
                    bb0, bb1 = int(tBOff[cb0]), int(tBOff[cb1])
                    nA, nB_ = a1 - a0, bb1 - bb0
                    g = gp.tile([P, (nA + nB_) * P], bf16, tag="g")
                    GT = 8  # max tiles (1024 idxs) per dma_gather instruction
                    for k in range(0, nA, GT):
                        w = min(GT, nA - k)
                        nc.gpsimd.dma_gather(
                            g[:, (k) * P:(k + w) * P].rearrange("p (t f) -> p t f", f=P),
                            tab[0:HALF, :], idxA_sb[:, (a0 + k) * 8:(a0 + k + w) * 8],
                            w * P, w * P, P)
                    for k in range(0, nB_, GT):
                        w = min(GT, nB_ - k)
                        nc.gpsimd.dma_gather(
                            g[:, (nA + k) * P:(nA + k + w) * P].rearrange("p (t f) -> p t f", f=P),
                            tab[HALF:R, :], idxB_sb[:, (bb0 + k) * 8:(bb0 + k + w) * 8],
                            w * P, w * P, P)

                    # one-hot group cache for this chunk
                    ohcache = {}

                    def get_oh(dc, rs, re):
                        base = rs + ((dc - rs) // K) * K
                        w = min(K, re - base)
                        if base not in ohcache:
                            t = ohp.tile([P, w * P], bf16, tag="oh")
                            nc.vector.tensor_tensor(
                                out=t[:].rearrange("p (a b) -> p a b", b=P),
                                in0=dl_sb[:, base:base + w].unsqueeze(2).to_broadcast([P, w, P]),
                                in1=iota_rep[:, :w, :],
                                op=mybir.AluOpType.is_equal)
                            ohcache[base] = t
                        t = ohcache[base]
                        sl = dc - base
                        return t[:, sl * P:(sl + 1) * P]

                    for b in range(cb0, cb1):
                        tiles = []
                        for ti in range(int(nTA[b])):
                            gcol = int(tAOff[b]) + ti - a0
                            dcol = int(tAOff[b]) + ti
                            tiles.append((gcol, dcol, a0, a1))
                        for ti in range(int(nTB[b])):
                            gcol = nA + int(tBOff[b]) + ti - bb0
                            dcol = TA + int(tBOff[b]) + ti
                            tiles.append((gcol, dcol, TA + bb0, TA + bb1))
                        ps = pseg.tile([P, P], f32, tag="ps")
                        nt = len(tiles)
                        for j, (gcol, dcol, rs, re) in enumerate(tiles):
                            oh = get_oh(dcol, rs, re)
                            nc.tensor.matmul(
                                out=ps[:], lhsT=g[:, gcol * P:(gcol + 1) * P],
                                rhs=oh, start=(j == 0), stop=(j == nt - 1))
                        nc.vector.tensor_tensor(
                            out=agg[:, b * P:(b + 1) * P], in0=ps[:],
                            in1=rc_sb[:, b * P:(b + 1) * P], op=mybir.AluOpType.mult)

                    # GEMMs for this chunk's columns
                    c_off = cb0 * P
                    c_w = (cb1 - cb0) * P
                    for off, w in ((c_off, c_w // 2), (c_off + c_w // 2, c_w - c_w // 2)):
                        pg = pgem.tile([P, 448], f32, tag="pg")
                        nc.tensor.matmul(out=pg[:fo, :w], lhsT=w_sb[f"wl{layer}"][:],
                                         rhs=agg[:, off:off + w], start=True, stop=False)
                        nc.tensor.matmul(out=pg[:fo, :w], lhsT=w_sb[f"wr{layer}"][:],
                                         rhs=xTc[:, off:off + w], start=False, stop=True)
                        if layer == 3:
                            nc.scalar.activation(out=hpre[:fo, off:off + w], in_=pg[:fo, :w],
                                                 func=AF.Identity, bias=b3_sb[:fo], scale=1.0)
                        else:
                            nc.scalar.activation(out=hpre[:, off:off + w], in_=pg[:, :w],
                                                 func=AF.Copy, bias=0.0, scale=1.0)

                if dbg_hpre and layer == n_layers:
                    nc.sync.dma_start(out=t_dbg[:], in_=hpre[:])
                    break
                if layer < 3:
                    li = layer - 1
                    s1 = mp.tile([P, 1], f32, tag="st")
                    nc.vector.tensor_reduce(out=s1[:], in_=hpre[:], axis=mybir.AxisListType.X,
                                            op=mybir.AluOpType.add)
                    s2 = mp.tile([P, 1], f32, tag="st")
                    nc.scalar.activation(out=agg[:], in_=hpre[:], func=AF.Square,
                                         bias=0.0, scale=1.0, accum_out=s2[:])
                    stt = mp.tile([P, 2], f32, tag="st2")
                    nc.vector.tensor_copy(out=stt[:, 0:1], in_=s1[:])
                    nc.vector.tensor_copy(out=stt[:, 1:2], in_=s2[:])
                    nc.sync.dma_start(out=st_in[li][:], in_=stt[:])
                    nc.gpsimd.collective_compute(
                        "AllReduce", mybir.AluOpType.add, replica_groups=RG,
                        ins=[st_in[li][:]], outs=[st_out[li][:]])
                    str_ = mp.tile([P, 2], f32, tag="st2")
                    nc.sync.dma_start(out=str_[:], in_=st_out[li][:])
                    mu = mp.tile([P, 1], f32, tag="st")
                    nc.scalar.mul(out=mu[:], in_=str_[:, 0:1], mul=1.0 / N)
                    ex2 = mp.tile([P, 1], f32, tag="st")
                    nc.scalar.mul(out=ex2[:], in_=str_[:, 1:2], mul=1.0 / N)
                    var = mp.tile([P, 1], f32, tag="st")
                    nc.vector.tensor_tensor(out=var[:], in0=mu[:], in1=mu[:], op=mybir.AluOpType.mult)
                    nc.vector.tensor_tensor(out=var[:], in0=ex2[:], in1=var[:], op=mybir.AluOpType.subtract)
                    rs_ = mp.tile([P, 1], f32, tag="st")
                    nc.scalar.activation(out=rs_[:], in_=var[:], func=AF.Sqrt,
                                         bias=eps_sb[:], scale=1.0, alpha=0.0)
                    nc.vector.reciprocal(out=rs_[:], in_=rs_[:])
                    a_t = mp.tile([P, 1], f32, tag="st")
                    nc.vector.tensor_tensor(out=a_t[:], in0=rs_[:], in1=bn_sb[:, 2 * li:2 * li + 1],
                                            op=mybir.AluOpType.mult)
                    bi = mp.tile([P, 1], f32, tag="st")
                    nc.vector.tensor_tensor(out=bi[:], in0=mu[:], in1=a_t[:], op=mybir.AluOpType.mult)
                    nc.vector.tensor_tensor(out=bi[:], in0=bn_sb[:, 2 * li + 1:2 * li + 2], in1=bi[:],
                                            op=mybir.AluOpType.subtract)
                    nc.scalar.activation(out=hbn[:], in_=hpre[:], func=AF.Relu,
                                         bias=bi[:], scale=a_t[:])
                    nc.vector.memset(hbn[:, NPC:PADC], 0.0)
                    for k in range(NB):
                        pt = ptr.tile([P, P], bf16, tag="pt")
                        nc.tensor.transpose(out=pt[:], in_=hbn[:, k * P:(k + 1) * P], identity=ident_bf[:])
                        ts_ = tsp.tile([P, P], bf16, tag="ts")
                        nc.scalar.activation(out=ts_[:], in_=pt[:], func=AF.Copy, bias=0.0, scale=1.0)
                        nc.sync.dma_start(out=h_own[li][k * P:(k + 1) * P, :], in_=ts_[:])
                    nc.gpsimd.collective_compute(
                        "AllGather", mybir.AluOpType.bypass, replica_groups=RG,
                        ins=[h_own[li][:]], outs=[h_tab[li][:]])
                else:
                    for k in range(NB):
                        pt = ptr.tile([P, P], f32, tag="pt")
                        nc.tensor.transpose(out=pt[:], in_=hpre[:, k * P:(k + 1) * P], identity=ident_f[:])
                        ts_ = tsp.tile([P, P], f32, tag="ts")
                        nc.scalar.activation(out=ts_[:], in_=pt[:], func=AF.Copy, bias=0.0, scale=1.0)
                        nc.sync.dma_start(out=t_out[k * P:(k + 1) * P, :], in_=ts_[:, :OUT_DIM])
    nc.compile()
    return nc


def kernel(**inputs):
    import os
    os.environ.setdefault("BASS_NEVER_TRACE", "1")
    import ml_dtypes
    from concourse.bass_utils import run_bass_kernel_spmd

    x = np.asarray(inputs["x"], dtype=np.float32)
    ei = np.asarray(inputs["edge_index"])
    idxA_w, idxB_w, dl_all, recip_b, x_table, xT_own, plan = _preprocess(x, ei)
    nc = _build(plan)

    bn = np.stack([np.asarray(inputs["g1"]), np.asarray(inputs["be1"]),
                   np.asarray(inputs["g2"]), np.asarray(inputs["be2"])], axis=1).astype(np.float32)
    b3 = np.zeros((P, 1), np.float32)
    b3[:OUT_DIM, 0] = np.asarray(inputs["b3"], dtype=np.float32)
    wm = {}
    for l, (wl, wr) in {1: ("Wl1", "Wr1"), 2: ("Wl2", "Wr2"), 3: ("Wl3", "Wr3")}.items():
        wm[f"wl{l}"] = np.ascontiguousarray(
            np.asarray(inputs[wl], dtype=np.float32).T).astype(ml_dtypes.bfloat16)
        wm[f"wr{l}"] = np.ascontiguousarray(
            np.asarray(inputs[wr], dtype=np.float32).T).astype(ml_dtypes.bfloat16)

    in_maps = []
    for c in range(NC_):
        m = {"x_table": x_table, "idxA": idxA_w[c], "idxB": idxB_w[c],
             "dl": dl_all[c], "recip": recip_b[c], "xT_own": xT_own[c],
             "bn": bn, "b3": b3}
        m.update(wm)
        in_maps.append(m)
    res = run_bass_kernel_spmd(nc, in_maps, core_ids=list(range(NC_)))
    global _last_res
    _last_res = res
    out = np.concatenate([res.results[c]["out"][:NPC] for c in range(NC_)], axis=0)
    return out.astype(np.float32)


_last_res = None
